# revision 9
# baseline (speedup 1.0000x reference)
"""Trainium2 Bass kernel for batched CRF Viterbi decode.

Problem: x [1024, 512, 41] f32 emissions, transitions [41,41], start/end [41].
Output: tags [1024, 512] int64 (Viterbi path, torchcrf semantics).

Strategy (pure data parallel over batch):
- 8 cores x 128 batch rows; batch -> SBUF partitions (128 exactly).
- Forward recurrence over T on the Vector engine, free dim = (j_next, i_prev)
  of size 41*41=1681:
    A = score_bcast + transT            (fp32 TT add, exact ref order)
    Cand = A + emit_bcast               (fp32 TT add)
    score' = reduce_max_inner(Cand)     (grouped reduce, axis=X)
    D = (Cand == score'_bcast)          (scalar_tensor_tensor is_equal)
    DW = D * w_bcast, w[i] = 41-i       (TT mult)
    bpE = reduce_max_inner(DW) = 41-argmax_first  (stored bf16, exact ints)
- Final: add end_transitions, same max/argmax trick.
- Backtrace: onehot dot bpE row (tensor_tensor_reduce) walks backpointers.
All adds replicate the fp32 rounding order of the jax reference exactly;
argmax ties resolve to the smallest index exactly like jnp.argmax.
"""

import ml_dtypes
import numpy as np

import concourse.bacc as bacc
import concourse.mybir as mybir
import concourse.tile as tile
from concourse.bass_utils import run_bass_kernel_spmd

B, T, C = 1024, 512, 41
NCORES = 8
BL = B // NCORES          # 128 batch rows per core = SBUF partitions
CC = C * C

F32 = mybir.dt.float32
BF16 = mybir.dt.bfloat16
ALU = mybir.AluOpType
AX = mybir.AxisListType


def emit_viterbi(tc, ins, outs, n_steps, gpsimd_bp=True):
    """Emit the Viterbi program. ins/outs: dicts of DRAM APs.

    ins: x [BL, n_steps*C] (emissions t=1..), score0 [BL, C],
         transT [BL, C*C] (transT[j*C+i] = trans[i,j]), w [BL, C],
         iota [BL, C], end [BL, C]
    outs: tags [BL, n_steps+1] f32

    gpsimd_bp: run the backpointer compare/mask (D, DW) on GPSIMD in
    parallel with the Vector-engine recurrence (double-buffered).
    """
    nc = tc.nc
    v = nc.vector
    g = nc.gpsimd if gpsimd_bp else nc.vector
    S = n_steps
    NB = 2  # parity buffers

    x_sb = nc.alloc_sbuf_tensor("x_sb", [BL, S * C], F32).ap()
    tT_sb = nc.alloc_sbuf_tensor("tT_sb", [BL, CC], F32).ap()
    w_sb = nc.alloc_sbuf_tensor("w_sb", [BL, C], F32).ap()
    iota_sb = nc.alloc_sbuf_tensor("iota_sb", [BL, C], BF16).ap()
    end_sb = nc.alloc_sbuf_tensor("end_sb", [BL, C], F32).ap()
    sc = [nc.alloc_sbuf_tensor(f"score{k}", [BL, C], F32).ap() for k in range(NB)]
    fin = nc.alloc_sbuf_tensor("fin", [BL, C], F32).ap()
    mfin = nc.alloc_sbuf_tensor("mfin", [BL, 1], F32).ap()
    A = nc.alloc_sbuf_tensor("A", [BL, CC], F32).ap()
    Cc = [nc.alloc_sbuf_tensor(f"Cc{k}", [BL, CC], F32).ap() for k in range(NB)]
    D = [nc.alloc_sbuf_tensor(f"D{k}", [BL, CC], F32).ap() for k in range(NB)]
    DW = [nc.alloc_sbuf_tensor(f"DW{k}", [BL, CC], F32).ap() for k in range(NB)]
    bpE = nc.alloc_sbuf_tensor("bpE", [BL, S * C], BF16).ap()
    tags_sb = nc.alloc_sbuf_tensor("tags_sb", [BL, S + 1], F32).ap()
    val = nc.alloc_sbuf_tensor("val", [BL, 1], F32).ap()
    oh = nc.alloc_sbuf_tensor("oh", [BL, C], BF16).ap()
    scr = nc.alloc_sbuf_tensor("scr", [BL, C], BF16).ap()

    # Load everything
    nc.sync.dma_start(out=x_sb, in_=ins["x"])
    nc.sync.dma_start(out=tT_sb, in_=ins["transT"])
    nc.sync.dma_start(out=w_sb, in_=ins["w"])
    nc.sync.dma_start(out=iota_sb, in_=ins["iota"])
    nc.sync.dma_start(out=end_sb, in_=ins["end"])
    nc.sync.dma_start(out=sc[0], in_=ins["score0"])

    tT3 = tT_sb.rearrange("p (j i) -> p j i", i=C)
    A3 = A.rearrange("p (j i) -> p j i", i=C)
    C3 = [t.rearrange("p (j i) -> p j i", i=C) for t in Cc]
    D3 = [t.rearrange("p (j i) -> p j i", i=C) for t in D]
    DW3 = [t.rearrange("p (j i) -> p j i", i=C) for t in DW]
    sc_rep = [t.unsqueeze(1).broadcast_to([BL, C, C]) for t in sc]   # over j
    M_rep = [t.unsqueeze(2).broadcast_to([BL, C, C]) for t in sc]    # over i
    w_rep = w_sb.unsqueeze(1).broadcast_to([BL, C, C])               # over j

    # Forward. Step s: score in sc[s%2], new score -> sc[(s+1)%2].
    # D/DW (backpointer extraction) are off the critical chain; with
    # gpsimd_bp they run on GPSIMD one step behind, and the DVE picks up
    # the grouped bpE reduce a step later.
    for s in range(S):
        p = s % NB
        q = (s + 1) % NB
        emit_rep = x_sb[:, s * C:(s + 1) * C].unsqueeze(2).broadcast_to([BL, C, C])
        v.tensor_tensor(out=A3, in0=sc_rep[p], in1=tT3, op=ALU.add)
        v.tensor_tensor(out=C3[p], in0=A3, in1=emit_rep, op=ALU.add)
        v.tensor_reduce(out=sc[q], in_=C3[p], axis=AX.X, op=ALU.max)
        g.tensor_tensor(out=D3[p], in0=C3[p], in1=M_rep[q], op=ALU.is_equal)
        g.tensor_tensor(out=DW3[p], in0=D3[p], in1=w_rep, op=ALU.mult)
        v.tensor_reduce(out=bpE[:, s * C:(s + 1) * C], in_=DW3[p], axis=AX.X, op=ALU.max)

    score = sc[S % NB]
    # Final step: add end transitions, argmax -> last tag
    v.tensor_tensor(out=fin, in0=score, in1=end_sb, op=ALU.add)
    v.tensor_reduce(out=mfin, in_=fin, axis=AX.X, op=ALU.max)
    v.tensor_scalar(out=D[0][:, :C], in0=fin, scalar1=mfin, scalar2=None, op0=ALU.is_equal)
    v.tensor_tensor(out=DW[0][:, :C], in0=D[0][:, :C], in1=w_sb, op=ALU.mult)
    v.tensor_reduce(out=val, in_=DW[0][:, :C], axis=AX.X, op=ALU.max)
    v.tensor_scalar(out=tags_sb[:, S:S + 1], in0=val, scalar1=float(C), scalar2=-1.0,
                    op0=ALU.subtract, op1=ALU.mult)
    v.tensor_scalar(out=oh, in0=iota_sb, scalar1=tags_sb[:, S:S + 1], scalar2=None,
                    op0=ALU.is_equal)

    # Backtrace
    for s in range(S - 1, -1, -1):
        v.tensor_tensor(out=scr, in0=oh, in1=bpE[:, s * C:(s + 1) * C], op=ALU.mult)
        v.tensor_reduce(out=val, in_=scr, axis=AX.X, op=ALU.add)
        v.tensor_scalar(out=tags_sb[:, s:s + 1], in0=val, scalar1=float(C), scalar2=-1.0,
                        op0=ALU.subtract, op1=ALU.mult)
        v.tensor_scalar(out=oh, in0=iota_sb, scalar1=tags_sb[:, s:s + 1], scalar2=None,
                        op0=ALU.is_equal)

    nc.sync.dma_start(out=outs["tags"], in_=tags_sb)


def host_inputs(x, start_transitions, end_transitions, transitions):
    """Host-side prep: per-core input dicts (all float32 numpy)."""
    x = np.asarray(x, dtype=np.float32)
    start = np.asarray(start_transitions, dtype=np.float32)
    end = np.asarray(end_transitions, dtype=np.float32)
    trans = np.asarray(transitions, dtype=np.float32)
    n_b, n_t, n_c = x.shape
    bl = n_b // NCORES
    S = n_t - 1

    score0 = start[None, :] + x[:, 0]                       # [B, C] exact fp32
    xr = np.ascontiguousarray(x[:, 1:, :].reshape(n_b, S * n_c))
    transT = np.ascontiguousarray(
        np.broadcast_to(trans.T.reshape(-1), (bl, n_c * n_c)))
    w = np.ascontiguousarray(
        np.broadcast_to((n_c - np.arange(n_c)).astype(np.float32), (bl, n_c)))
    iota = np.ascontiguousarray(
        np.broadcast_to(np.arange(n_c, dtype=np.float32), (bl, n_c))
    ).astype(ml_dtypes.bfloat16)
    end_b = np.ascontiguousarray(np.broadcast_to(end, (bl, n_c)))

    in_maps = []
    for c in range(NCORES):
        sl = slice(c * bl, (c + 1) * bl)
        in_maps.append(dict(
            x=np.ascontiguousarray(xr[sl]),
            score0=np.ascontiguousarray(score0[sl]),
            transT=transT, w=w, iota=iota, end=end_b,
        ))
    return in_maps


_PROGRAM_CACHE = {}


def build_program(n_steps=T - 1):
    key = n_steps
    if key in _PROGRAM_CACHE:
        return _PROGRAM_CACHE[key]
    nc = bacc.Bacc("TRN2", target_bir_lowering=False, debug=False,
                   num_devices=NCORES)
    ins = dict(
        x=nc.dram_tensor("x", [BL, n_steps * C], F32, kind="ExternalInput").ap(),
        score0=nc.dram_tensor("score0", [BL, C], F32, kind="ExternalInput").ap(),
        transT=nc.dram_tensor("transT", [BL, CC], F32, kind="ExternalInput").ap(),
        w=nc.dram_tensor("w", [BL, C], F32, kind="ExternalInput").ap(),
        iota=nc.dram_tensor("iota", [BL, C], BF16, kind="ExternalInput").ap(),
        end=nc.dram_tensor("end", [BL, C], F32, kind="ExternalInput").ap(),
    )
    outs = dict(
        tags=nc.dram_tensor("tags", [BL, n_steps + 1], F32,
                            kind="ExternalOutput").ap(),
    )
    with tile.TileContext(nc) as tc:
        emit_viterbi(tc, ins, outs, n_steps)
    nc.compile()
    _PROGRAM_CACHE[key] = nc
    return nc


def kernel(x, start_transitions, end_transitions, transitions):
    nc = build_program(T - 1)
    in_maps = host_inputs(x, start_transitions, end_transitions, transitions)
    res = run_bass_kernel_spmd(nc, in_maps, core_ids=list(range(NCORES)))
    tags = np.concatenate([res.results[c]["tags"] for c in range(NCORES)], axis=0)
    return tags.astype(np.int64)
